# revision 1
# baseline (speedup 1.0000x reference)
"""CGNN layer kernel for Trainium2 (8 NeuronCores, SPMD).

Sharding: core c owns batch b = c//2 and receiver-node half i0 = (c%2)*128.
Each core computes its (128, 128) output shard from full-j message passing.

Math (per core, b fixed):
  z[i,j,:]  = W1a x_i + W1b x_j + W1d a_ij + W1c c + b1        (pre-activation)
  s[i,:]    = sum_j mask_j * silu(z[i,j,:])
  aggr      = W2 s + b2 * (#live j)
  u         = silu(W3 [x, aggr] + b3); out = LN(x + W4 u + b4) * gamma + beta

Device layout: z kept as (h=128 partitions, j=256 free) per receiver i.
  - adj term: PE-transpose 4-receiver stacks of adj (j,r)->(r,j), masked evict,
    then K=32 row-tiled matmuls (tile_position) against replicated W1d^T.
  - x_j term: one K=128 matmul vs pre-masked x^T (same operands every i).
  - bias+silu+sum_j: single ACT op (bias port + accum_out).
  - masked-j bias pollution removed in closed form: s -= nm0 * silu(beta_i).

Scheduling notes: walrus gives compute instructions a budget of ONE semaphore
wait, and only waits arising from real data dependencies update Tile's
per-engine clock. The kernel therefore "absorbs" cross-engine production ticks
with tiny 1x1 matmuls that genuinely read one stale element of the producer
tile (into a dedicated PSUM scratch column), so every real matmul needs at
most its single PSUM-recycle wait. All MLP biases are folded into PSUM via
K=1 rank-1 matmuls of host-provided bias ROWS against a ones row, so no ACT
instruction ever waits on a DMA. All PSUM pools live for the whole program so
banks never alias across phases.
"""

import numpy as np
import ml_dtypes
ml_bf16 = ml_dtypes.bfloat16
from contextlib import ExitStack

import concourse.bass as bass
import concourse.bacc as bacc
import concourse.mybir as mybir
import concourse.tile as tile
from concourse.bass_utils import run_bass_kernel_spmd
from concourse.tile_rust import add_dep_helper

B, N, H, R = 4, 256, 128, 32
NI = 128          # receivers per core
NQ = NI // 4      # receiver quads
FP = mybir.dt.float32
BF = mybir.dt.bfloat16
EPS = 1e-5
ALU = mybir.AluOpType
ACTF = mybir.ActivationFunctionType

_cache = {}


def _order(later, earlier):
    a = later.ins if hasattr(later, "ins") else later
    b = earlier.ins if hasattr(earlier, "ins") else earlier
    add_dep_helper(a, b, sync=False, reason="pe order")


def _build_program():
    nc = bacc.Bacc()

    # ---- per-core DRAM parameters ----
    adj = nc.declare_dram_parameter("adj", [NI, N, R], FP, isOutput=False)
    x_all = nc.declare_dram_parameter("x_all", [N, H], FP, isOutput=False)
    xi = nc.declare_dram_parameter("xi", [NI, H], FP, isOutput=False)
    maskf = nc.declare_dram_parameter("maskf", [N], FP, isOutput=False)
    condrep = nc.declare_dram_parameter("condrep", [2 * H, H], FP, isOutput=False)
    w1aT = nc.declare_dram_parameter("w1aT", [H, H], FP, isOutput=False)
    w1bT = nc.declare_dram_parameter("w1bT", [H, H], BF, isOutput=False)
    w1cT = nc.declare_dram_parameter("w1cT", [2 * H, H], FP, isOutput=False)
    w1dTrep = nc.declare_dram_parameter("w1dTrep", [H, H], BF, isOutput=False)
    w2T = nc.declare_dram_parameter("w2T", [H, H], FP, isOutput=False)
    w3aT = nc.declare_dram_parameter("w3aT", [H, H], FP, isOutput=False)
    w3bT = nc.declare_dram_parameter("w3bT", [H, H], FP, isOutput=False)
    w4T = nc.declare_dram_parameter("w4T", [H, H], FP, isOutput=False)
    b1row = nc.declare_dram_parameter("b1row", [1, H], FP, isOutput=False)
    b2row = nc.declare_dram_parameter("b2row", [1, H], FP, isOutput=False)
    b3row = nc.declare_dram_parameter("b3row", [1, H], FP, isOutput=False)
    b4row = nc.declare_dram_parameter("b4row", [1, H], FP, isOutput=False)
    onesrow = nc.declare_dram_parameter("onesrow", [1, NI], FP, isOutput=False)
    identp = nc.declare_dram_parameter("identp", [H, H], FP, isOutput=False)
    gamma_rep = nc.declare_dram_parameter("gamma_rep", [H, H], FP, isOutput=False)
    beta_rep = nc.declare_dram_parameter("beta_rep", [H, H], FP, isOutput=False)
    out = nc.declare_dram_parameter("out", [NI, H], FP, isOutput=True)

    with ExitStack() as ctx:
        tc = ctx.enter_context(tile.TileContext(nc))
        const = ctx.enter_context(tc.tile_pool(name="const", bufs=1))
        persist = ctx.enter_context(tc.tile_pool(name="persist", bufs=1))
        work = ctx.enter_context(tc.tile_pool(name="work", bufs=2))
        adjbuf = ctx.enter_context(tc.tile_pool(name="adjbuf", bufs=3))
        scr = ctx.enter_context(tc.tile_pool(name="scr", bufs=3))
        # PSUM: 2 (setup/epilogue) + 4 (z) + 2 (adjT)
        pep = ctx.enter_context(tc.tile_pool(name="pep", bufs=2, space="PSUM"))
        pz = ctx.enter_context(tc.tile_pool(name="pz", bufs=4, space="PSUM"))
        pt = ctx.enter_context(tc.tile_pool(name="pt", bufs=2, space="PSUM"))

        cload_tiles = []

        def cload(ap, shape, tag, dt=FP):
            if not isinstance(ap, bass.AP):
                ap = ap[:]
            t = const.tile(shape, dt, tag=tag, name=tag)
            nc.sync.dma_start(out=t, in_=ap)
            cload_tiles.append(t)
            return t

        ident_sb = cload(identp, [H, H], "ident")
        w1aT_sb = cload(w1aT, [H, H], "w1aT")
        w1bT_sb = cload(w1bT, [H, H], "w1bT", dt=BF)
        w1cT_sb0 = cload(w1cT[0:H, :], [H, H], "w1cT0")
        w1cT_sb1 = cload(w1cT[H:2 * H, :], [H, H], "w1cT1")
        w1dTrep_sb = cload(w1dTrep, [H, H], "w1dTrep", dt=BF)
        w2T_sb = cload(w2T, [H, H], "w2T")
        w3aT_sb = cload(w3aT, [H, H], "w3aT")
        w3bT_sb = cload(w3bT, [H, H], "w3bT")
        w4T_sb = cload(w4T, [H, H], "w4T")
        condrep_sb0 = cload(condrep[0:H, :], [H, H], "condrep0")
        condrep_sb1 = cload(condrep[H:2 * H, :], [H, H], "condrep1")
        b1r_sb = cload(b1row, [1, H], "b1r")
        b2r_sb = cload(b2row, [1, H], "b2r")
        b3r_sb = cload(b3row, [1, H], "b3r")
        b4r_sb = cload(b4row, [1, H], "b4r")
        ones_sb = cload(onesrow, [1, NI], "onesr")
        xi_sb = cload(xi, [NI, H], "xi")
        xall_sb0 = cload(x_all[0:H, :], [H, H], "xall0")
        xall_sb1 = cload(x_all[H:N, :], [H, H], "xall1")
        gamma_sb = cload(gamma_rep, [H, H], "gamma_rep")
        beta_sb = cload(beta_rep, [H, H], "beta_rep")

        # mask broadcast to all partitions: (128, 256)
        maskrep = persist.tile([H, N], FP, tag="maskrep", name="maskrep")
        maskf_ap = maskf[:]
        mask_bcast = bass.AP(tensor=maskf_ap.tensor, offset=maskf_ap.offset,
                             ap=[[0, H]] + list(maskf_ap.ap))
        nc.sync.dma_start(out=maskrep, in_=mask_bcast)

        # per-partition live-count and masked-out-count of senders
        msum = persist.tile([H, 1], FP, tag="msum", name="msum")
        mrow_scr = persist.tile([H, N], FP, tag="mrow_scr", name="mrow_scr")
        nc.vector.tensor_scalar(mrow_scr, maskrep, 1.0, None,
                                ALU.mult, ALU.add, accum_out=msum)
        nm0col = persist.tile([H, 1], FP, tag="nm0col", name="nm0col")
        nc.vector.tensor_scalar(nm0col, msum, -1.0, float(N), ALU.mult, ALU.add)
        # msum replicated as a row (all partitions of msum hold the same value)
        msum_row = persist.tile([1, NI], FP, tag="msum_row", name="msum_row")
        nc.vector.tensor_scalar(msum_row, ones_sb, msum[0:1, 0:1], None,
                                ALU.mult)

        xTm = persist.tile([H, N], BF, tag="xTm", name="xTm")
        xTi = persist.tile([H, NI], FP, tag="xTi", name="xTi")
        ACb = persist.tile([H, NI], FP, tag="ACb", name="ACb")
        siluAC = persist.tile([H, NI], FP, tag="siluAC", name="siluAC")
        korr = persist.tile([H, NI], FP, tag="korr", name="korr")
        S_raw = persist.tile([H, NI], FP, tag="S_raw", name="S_raw")

        # ---- setup: x transposes, ACb ----
        for half, xall_h in ((0, xall_sb0), (1, xall_sb1)):
            pxt = pep.tile([H, H], FP, tag="ps", name="pxt")
            nc.tensor.transpose(pxt, xall_h, ident_sb)
            nc.vector.scalar_tensor_tensor(
                out=xTm[:, half * H:(half + 1) * H], in0=pxt, scalar=1.0,
                in1=maskrep[:, half * H:(half + 1) * H],
                op0=ALU.mult, op1=ALU.mult)

        pxi = pep.tile([H, H], FP, tag="ps", name="pxi")
        nc.tensor.transpose(pxi, xi_sb, ident_sb)
        nc.vector.tensor_copy(xTi, pxi)

        # ACb = W1a x_i + W1c c + b1  -> (128 h, 128 i)
        pA = pep.tile([H, NI], FP, tag="ps", name="pA")
        nc.tensor.matmul(pA, lhsT=w1aT_sb, rhs=xTi, start=True, stop=False)
        nc.tensor.matmul(pA, lhsT=w1cT_sb0, rhs=condrep_sb0,
                         start=False, stop=False)
        nc.tensor.matmul(pA, lhsT=w1cT_sb1, rhs=condrep_sb1,
                         start=False, stop=False)
        nc.tensor.matmul(pA, lhsT=b1r_sb, rhs=ones_sb,
                         start=False, stop=True)
        nc.scalar.activation(ACb, pA, ACTF.Copy)

        # korr[h,i] = nm0 * silu(ACb[h,i])
        nc.scalar.activation(siluAC, ACb, ACTF.Silu)
        nc.vector.tensor_scalar(korr, siluAC, nm0col, None, ALU.mult)

        # ---- main loop over receiver quads ----
        stacks = persist.tile([H, NQ, 2, 4, R], FP, tag="stacks",
                              name="stacks")
        for q in range(NQ):
            st0 = stacks[:, q, 0]
            st1 = stacks[:, q, 1]
            for jt, st, eng in ((0, st0, nc.sync), (1, st1, nc.scalar)):
                asrc = adj[4 * q:4 * q + 4, jt * H:(jt + 1) * H, :]
                eng.dma_start(out=st, in_=asrc.rearrange("g j r -> j g r"))

            ptile = pt.tile([H, N], FP, tag="ptile", name="ptile")
            nc.tensor.transpose(
                ptile[:, 0:H], st0.rearrange("j g r -> j (g r)"), ident_sb)
            nc.tensor.transpose(
                ptile[:, H:N], st1.rearrange("j g r -> j (g r)"), ident_sb)

            atile = adjbuf.tile([H, N], BF, tag="atile", name="atile")
            nc.vector.scalar_tensor_tensor(
                out=atile, in0=ptile, scalar=1.0, in1=maskrep,
                op0=ALU.mult, op1=ALU.mult)

            zts = []
            for g in range(4):
                zt = pz.tile([H, N], FP, tag="zt", name="zt")
                nc.tensor.matmul(zt, lhsT=w1bT_sb, rhs=xTm,
                                 start=True, stop=False)
                zts.append(zt)
            for g in range(4):
                nc.tensor.matmul(
                    zts[g], lhsT=w1dTrep_sb[32 * g:32 * g + 32, :],
                    rhs=atile[32 * g:32 * g + 32, :],
                    start=False, stop=True, tile_position=(32 * g, 0))
            for g in range(4):
                li = 4 * q + g
                sct = scr.tile([H, N], BF, tag="sct", name="sct")
                nc.scalar.activation(sct, zts[g], ACTF.Silu,
                                     bias=ACb[:, li:li + 1])
                sink = scr.tile([H, N], BF, tag="sink", name="sink")
                nc.vector.tensor_scalar(sink, sct, 1.0, None, ALU.mult,
                                        ALU.add, accum_out=S_raw[:, li:li + 1])

        # ---- epilogue ----
        S_true = persist.tile([H, NI], FP, tag="S_true", name="S_true")
        nc.vector.scalar_tensor_tensor(out=S_true, in0=S_raw, scalar=0.0,
                                       in1=korr, op0=ALU.add,
                                       op1=ALU.subtract)
        # aggr = W2 s + b2 * live_count
        pa = pep.tile([H, NI], FP, tag="ps", name="pa")
        nc.tensor.matmul(pa, lhsT=w2T_sb, rhs=S_true, start=True, stop=False)
        nc.tensor.matmul(pa, lhsT=b2r_sb, rhs=msum_row, start=False,
                         stop=True)
        aggrT = work.tile([H, NI], FP, tag="aggrT", name="aggrT")
        nc.scalar.activation(aggrT, pa, ACTF.Copy)

        pu = pep.tile([H, NI], FP, tag="ps", name="pu")
        nc.tensor.matmul(pu, lhsT=w3aT_sb, rhs=xTi, start=True, stop=False)
        nc.tensor.matmul(pu, lhsT=w3bT_sb, rhs=aggrT, start=False, stop=False)
        nc.tensor.matmul(pu, lhsT=b3r_sb, rhs=ones_sb, start=False,
                         stop=True)
        u_sb = work.tile([H, NI], FP, tag="u_sb", name="u_sb")
        nc.scalar.activation(u_sb, pu, ACTF.Silu)

        pupd = pep.tile([H, NI], FP, tag="ps", name="pupd")
        nc.tensor.matmul(pupd, lhsT=w4T_sb, rhs=u_sb, start=True, stop=False)
        nc.tensor.matmul(pupd, lhsT=b4r_sb, rhs=ones_sb, start=False,
                         stop=True)
        updT = work.tile([H, NI], FP, tag="updT", name="updT")
        nc.scalar.activation(updT, pupd, ACTF.Copy)

        py = pep.tile([NI, H], FP, tag="ps", name="py")
        nc.tensor.transpose(py, updT, ident_sb)

        y_sb = work.tile([NI, H], FP, tag="y_sb", name="y_sb")
        rowsum = work.tile([NI, 1], FP, tag="rowsum", name="rowsum")
        nc.vector.scalar_tensor_tensor(out=y_sb, in0=py, scalar=0.0,
                                       in1=xi_sb, op0=ALU.add, op1=ALU.add,
                                       accum_out=rowsum)
        negmu = work.tile([NI, 1], FP, tag="negmu", name="negmu")
        nc.vector.tensor_scalar(negmu, rowsum, -1.0 / H, None, ALU.mult)

        ysq = work.tile([NI, H], FP, tag="ysq", name="ysq")
        sumsq = work.tile([NI, 1], FP, tag="sumsq", name="sumsq")
        nc.vector.scalar_tensor_tensor(out=ysq, in0=y_sb, scalar=0.0,
                                       in1=y_sb, op0=ALU.add, op1=ALU.mult,
                                       accum_out=sumsq)
        # var + eps = sumsq/H - mu^2 + eps
        ex2 = work.tile([NI, 1], FP, tag="ex2", name="ex2")
        nc.vector.tensor_scalar(ex2, sumsq, 1.0 / H, float(EPS),
                                ALU.mult, ALU.add)
        musq = work.tile([NI, 1], FP, tag="musq", name="musq")
        nc.vector.scalar_tensor_tensor(out=musq, in0=negmu, scalar=0.0,
                                       in1=negmu, op0=ALU.add, op1=ALU.mult)
        vare = work.tile([NI, 1], FP, tag="vare", name="vare")
        nc.vector.scalar_tensor_tensor(out=vare, in0=ex2, scalar=0.0,
                                       in1=musq, op0=ALU.add,
                                       op1=ALU.subtract)
        sd = work.tile([NI, 1], FP, tag="sd", name="sd")
        nc.scalar.activation(sd, vare, ACTF.Sqrt)
        rstd = work.tile([NI, 1], FP, tag="rstd", name="rstd")
        nc.vector.reciprocal(rstd, sd)

        yn = work.tile([NI, H], FP, tag="yn", name="yn")
        nc.vector.tensor_scalar(yn, y_sb, negmu, rstd, ALU.add, ALU.mult)
        yg = work.tile([NI, H], FP, tag="yg", name="yg")
        nc.vector.scalar_tensor_tensor(out=yg, in0=yn, scalar=0.0,
                                       in1=gamma_sb, op0=ALU.add,
                                       op1=ALU.mult)
        yfin = work.tile([NI, H], FP, tag="yfin", name="yfin")
        nc.vector.scalar_tensor_tensor(out=yfin, in0=yg, scalar=0.0,
                                       in1=beta_sb, op0=ALU.add,
                                       op1=ALU.add)
        nc.sync.dma_start(out=out[:], in_=yfin)

    nc.finalize()
    return nc


def _get_program():
    if "nc" not in _cache:
        _cache["nc"] = _build_program()
    return _cache["nc"]


def kernel(x, adj_dist, mask, cond_vec, W1, b1, W2, b2, W3, b3, W4, b4,
           gamma, beta):
    x = np.asarray(x, dtype=np.float32)
    adj_dist = np.asarray(adj_dist, dtype=np.float32)
    mask_np = np.asarray(mask)
    cond_vec = np.asarray(cond_vec, dtype=np.float32)
    W1 = np.asarray(W1, dtype=np.float32)
    W2 = np.asarray(W2, dtype=np.float32)
    W3 = np.asarray(W3, dtype=np.float32)
    W4 = np.asarray(W4, dtype=np.float32)

    def c(a):
        return np.ascontiguousarray(a, dtype=np.float32)

    shared = dict(
        w1aT=c(W1[:, 0:H].T),
        w1bT=np.ascontiguousarray(W1[:, H:2 * H].T.astype(ml_bf16)),
        w1cT=c(W1[:, 2 * H + R:].T),
        w1dTrep=np.ascontiguousarray(
            np.tile(W1[:, 2 * H:2 * H + R].T, (4, 1)).astype(ml_bf16)),
        w2T=c(W2.T), w3aT=c(W3[:, 0:H].T), w3bT=c(W3[:, H:2 * H].T),
        w4T=c(W4.T),
        b1row=c(np.asarray(b1).reshape(1, H)),
        b2row=c(np.asarray(b2).reshape(1, H)),
        b3row=c(np.asarray(b3).reshape(1, H)),
        b4row=c(np.asarray(b4).reshape(1, H)),
        onesrow=c(np.ones((1, NI))),
        identp=c(np.eye(H)),
        gamma_rep=c(np.tile(np.asarray(gamma)[None, :], (H, 1))),
        beta_rep=c(np.tile(np.asarray(beta)[None, :], (H, 1))),
    )

    in_maps = []
    for core in range(8):
        b, ih = core // 2, core % 2
        i0 = ih * NI
        m = dict(shared)
        m["adj"] = c(adj_dist[b, i0:i0 + NI])
        m["x_all"] = c(x[b])
        m["xi"] = c(x[b, i0:i0 + NI])
        m["maskf"] = c(mask_np[b].astype(np.float32))
        m["condrep"] = c(np.tile(cond_vec[b][:, None], (1, H)))
        in_maps.append(m)

    nc = _get_program()
    _cache["in_maps"] = in_maps
    res = run_bass_kernel_spmd(nc, in_maps, list(range(8)))

    out_full = np.empty((B, N, H), dtype=np.float32)
    for core in range(8):
        b, ih = core // 2, core % 2
        out_full[b, ih * NI:(ih + 1) * NI] = res.results[core]["out"]
    return out_full

